# revision 5
# baseline (speedup 1.0000x reference)
"""Causal self-attention (B=4, T=2048, D=1024, H=16) on 8 trn2 NeuronCores.

Sharding: core c handles batch c//2 and head-group c%2 (8 heads each).
Each core computes qkv projections for its heads, attention, and a partial
output projection (its heads' columns of w_out). Host sums the two partial
outputs per batch and adds b_out.

Layout strategy (all matmuls fp32r, out = lhsT.T @ rhs):
  - host pre-transposes x and the weight slices so every operand is
    contraction-major in DRAM; no on-device transposes are needed.
  - scores are computed transposed: S.T[tk, tq] = kT_tile.T @ qT_tile,
    so exp(S.T) chunks feed the AV matmul directly as the moving operand.
  - the denominator rides along as a fused ones-column in the V stationary
    operand: psum_av[64] = sum_tk P.T = softmax denominator.
  - causal masking: key-block j is skipped entirely when fully masked for a
    query group; diagonal blocks get an additive -1e30 triangular pattern on
    PSUM before exp.
"""

import sys

sys.path.insert(0, "/opt/trn_rl_repo")

import numpy as np

D_MODEL = 1024
N_HEADS = 16
HEAD_DIM = 64
B, T = 4, 2048
N_CORES = 8
HG = 8          # heads per core
FH = HG * HEAD_DIM  # 512 features per core
TG = 512        # query group size
TK = 128        # key tile size
NG = T // TG    # 4 query groups
NK = T // TK    # 16 key tiles
NDC = D_MODEL // 128  # 8 contraction chunks
NEG = -1.0e30


def _build_program(mode, with_qkv_bias):
    """mode: 'causal' (tril mask), 'full' (all-ones mask), 'generic' (mask data)."""
    import concourse.bass as bass
    import concourse.mybir as mybir
    import concourse.tile as tile
    from concourse import bacc
    from concourse.bass import ts

    F32 = mybir.dt.float32
    F32R = mybir.dt.float32r
    EXP = mybir.ActivationFunctionType.Exp
    COPY = mybir.ActivationFunctionType.Copy
    MULT = mybir.AluOpType.mult
    ADD = mybir.AluOpType.add

    nc = bacc.Bacc("TRN2", target_bir_lowering=False, debug=False,
                   num_devices=N_CORES)

    xT = nc.declare_dram_parameter("xT", [D_MODEL, T], F32R, isOutput=False)
    wq = nc.declare_dram_parameter("wqT", [D_MODEL, FH], F32R, isOutput=False)
    wk = nc.declare_dram_parameter("wkT", [D_MODEL, FH], F32R, isOutput=False)
    wv = nc.declare_dram_parameter("wvT", [D_MODEL, FH], F32R, isOutput=False)
    wo = nc.declare_dram_parameter("woT", [FH, D_MODEL], F32R, isOutput=False)
    tri = None
    mneg = None
    if mode == "causal":
        tri = nc.declare_dram_parameter("trineg", [TK, NG * TG // 1], F32,
                                        isOutput=False)  # [128, 2048]
    elif mode == "generic":
        mneg = nc.declare_dram_parameter("maskTneg", [T, T], F32, isOutput=False)
    bqk = None
    bvb = None
    if with_qkv_bias:
        bqk = nc.declare_dram_parameter("bqk", [128, 8], F32, isOutput=False)
        bvb = nc.declare_dram_parameter("bvb", [128, FH], F32, isOutput=False)
    outT = nc.declare_dram_parameter("outT", [D_MODEL, T], F32, isOutput=True)

    xT_r = xT[:].rearrange("(c p) t -> p c t", p=128)
    wq_r = wq[:].rearrange("(c p) f -> p c f", p=128)
    wk_r = wk[:].rearrange("(c p) f -> p c f", p=128)
    wv_r = wv[:].rearrange("(c p) f -> p c f", p=128)
    wo_r = wo[:].rearrange("(c p) f -> p c f", p=128)

    with tile.TileContext(nc) as tc:
        with tc.tile_pool(name="persist", bufs=1) as persist, \
             tc.tile_pool(name="qkv", bufs=1) as qkv_pool:

            # persistent tensors for the attention phase
            q_t = [persist.tile([128, T], F32R, tag=f"q{fc}", name=f"q{fc}") for fc in range(4)]
            k_t = [persist.tile([128, T], F32R, tag=f"k{fc}", name=f"k{fc}") for fc in range(4)]
            vaug = [persist.tile([128, HG, HEAD_DIM + 1], F32R, tag=f"v{j}", name=f"v{j}")
                    for j in range(NK)]
            tri_t = None
            if mode == "causal":
                tri_t = persist.tile([TK, 2048], F32, tag="tri", name="tri")
                nc.sync.dma_start(out=tri_t[:], in_=tri[:])
            bqk_t = bvb_t = None
            if with_qkv_bias:
                bqk_t = persist.tile([128, 8], F32, tag="bqk", name="bqk")
                nc.sync.dma_start(out=bqk_t[:], in_=bqk[:])
                bvb_t = persist.tile([128, FH], F32, tag="bvb", name="bvb")
                nc.sync.dma_start(out=bvb_t[:], in_=bvb[:])

            # ---------------- projection phase ----------------
            with tc.tile_pool(name="projw", bufs=1) as projw, \
                 tc.tile_pool(name="xs", bufs=2) as xs, \
                 tc.tile_pool(name="pp", bufs=2, space="PSUM") as pp:
                wq_t = projw.tile([128, NDC, FH], F32R, tag="wq", name="wq")
                nc.sync.dma_start(out=wq_t[:], in_=wq_r)
                wk_t = projw.tile([128, NDC, FH], F32R, tag="wk", name="wk")
                nc.sync.dma_start(out=wk_t[:], in_=wk_r)
                wv_t = projw.tile([128, NDC, FH], F32R, tag="wv", name="wv")
                nc.sync.dma_start(out=wv_t[:], in_=wv_r)

                for tg in range(NG):
                    xt = xs.tile([128, NDC, TG], F32R, tag="xt", name="xt")
                    nc.sync.dma_start(out=xt[:], in_=xT_r[:, :, ts(tg, TG)])
                    # q and k: weights stationary -> feature-major outputs
                    for which, w_t, dst, boff in (("q", wq_t, q_t, 0),
                                                  ("k", wk_t, k_t, 4)):
                        for fc in range(4):
                            ps_ = pp.tile([128, TG], F32, tag="pp", name="ppt")
                            for dc in range(NDC):
                                nc.tensor.matmul(
                                    ps_[:], w_t[:, dc, ts(fc, 128)],
                                    xt[:, dc, :],
                                    start=(dc == 0), stop=(dc == NDC - 1))
                            if with_qkv_bias:
                                nc.scalar.activation(
                                    out=dst[fc][:, ts(tg, TG)], in_=ps_[:],
                                    func=mybir.ActivationFunctionType.Identity,
                                    bias=bqk_t[:, boff + fc:boff + fc + 1],
                                    scale=1.0)
                            else:
                                nc.scalar.copy(dst[fc][:, ts(tg, TG)], ps_[:])
                    # v: activations stationary -> token-major output
                    for tt in range(4):
                        j = tg * 4 + tt
                        ps_ = pp.tile([128, FH], F32, tag="pp", name="ppt")
                        for dc in range(NDC):
                            nc.tensor.matmul(
                                ps_[:], xt[:, dc, ts(tt, 128)], wv_t[:, dc, :],
                                start=(dc == 0), stop=(dc == NDC - 1))
                        if with_qkv_bias:
                            nc.vector.tensor_tensor(out=ps_[:], in0=ps_[:],
                                                    in1=bvb_t[:], op=ADD)
                        nc.scalar.copy(vaug[j][:, :, 0:HEAD_DIM], ps_[:])
                        nc.scalar.activation(out=vaug[j][:, :, HEAD_DIM:HEAD_DIM + 1],
                                             in_=wq_t[:, 0, 0:HG],
                                             func=COPY, scale=0.0, bias=1.0)

            # ---------------- attention phase ----------------
            with tc.tile_pool(name="wout", bufs=1) as woutp, \
                 tc.tile_pool(name="mgen", bufs=2) as mgen, \
                 tc.tile_pool(name="ppool", bufs=3) as ppool, \
                 tc.tile_pool(name="ypool", bufs=2) as ypool, \
                 tc.tile_pool(name="opool", bufs=3) as opool, \
                 tc.tile_pool(name="small", bufs=3) as small, \
                 tc.tile_pool(name="sp", bufs=2, space="PSUM") as sp, \
                 tc.tile_pool(name="avp", bufs=2, space="PSUM") as avp, \
                 tc.tile_pool(name="bcp", bufs=2, space="PSUM") as bcp, \
                 tc.tile_pool(name="pop", bufs=2, space="PSUM") as pop:

                wo_t = woutp.tile([128, 4, D_MODEL], F32R, tag="wo", name="wo")
                nc.sync.dma_start(out=wo_t[:], in_=wo_r)
                ones_r = woutp.tile([1, HEAD_DIM], F32R, tag="ones", name="ones")
                nc.scalar.activation(out=ones_r[:], in_=wo_t[0:1, 0, 0:HEAD_DIM],
                                     func=COPY, scale=0.0, bias=1.0)

                for g in range(NG):
                    jmax = 4 * g + 3 if mode == "causal" else NK - 1
                    mg_t = None
                    if mode == "generic":
                        mg_t = mgen.tile([128, NK, TG], F32, tag="mg", name="mg")
                        nc.sync.dma_start(
                            out=mg_t[:],
                            in_=mneg[:].rearrange("(j p) t -> p j t", p=128)
                            [:, :, ts(g, TG)])
                    y_t = [ypool.tile([128, TG], F32R, tag=f"y{fc}", name=f"y{fc}")
                           for fc in range(4)]
                    for h in range(HG):
                        fc, row = h // 2, (h % 2) * HEAD_DIM
                        pav = avp.tile([HEAD_DIM + 1, TG], F32, tag="pav", name="pav")
                        for j in range(jmax + 1):
                            ps_ = sp.tile([128, TG], F32, tag="ps", name="pst")
                            nc.tensor.matmul(
                                ps_[:],
                                k_t[fc][row:row + HEAD_DIM, ts(j, TK)],
                                q_t[fc][row:row + HEAD_DIM, ts(g, TG)],
                                start=True, stop=True)
                            if mode == "causal" and j >= 4 * g:
                                r = j - 4 * g
                                nc.vector.tensor_tensor(
                                    out=ps_[:], in0=ps_[:],
                                    in1=tri_t[:, ts(r, TG)], op=ADD)
                            elif mode == "generic":
                                nc.vector.tensor_tensor(
                                    out=ps_[:], in0=ps_[:],
                                    in1=mg_t[:, j, :], op=ADD)
                            p_t = ppool.tile([128, TG], F32R, tag="pt", name="pt")
                            nc.scalar.activation(out=p_t[:], in_=ps_[:],
                                                 func=EXP, scale=0.125)
                            nc.tensor.matmul(
                                pav[:], vaug[j][:, h, :], p_t[:],
                                start=(j == 0), stop=(j == jmax),
                                skip_group_check=True)
                        # normalize: O.T rows scaled by 1/denom (broadcast by
                        # a K=1 ones matmul across the 64 partitions)
                        rec = small.tile([1, TG], F32, tag="rec", name="rec")
                        nc.vector.reciprocal(rec[:], pav[HEAD_DIM:HEAD_DIM + 1, :])
                        rec_r = small.tile([1, TG], F32R, tag="recr", name="recr")
                        nc.scalar.copy(rec_r[:], rec[:])
                        pbc = bcp.tile([HEAD_DIM, TG], F32, tag="pbc", name="pbc")
                        nc.tensor.matmul(pbc[:], ones_r[:], rec_r[:],
                                         start=True, stop=True)
                        oraw = small.tile([HEAD_DIM, TG], F32, tag="oraw", name="oraw")
                        nc.scalar.copy(oraw[:], pav[0:HEAD_DIM, :])
                        nc.vector.tensor_tensor(
                            out=y_t[fc][row:row + HEAD_DIM, :],
                            in0=oraw[:], in1=pbc[:], op=MULT)
                    # output projection for this query group
                    for do in range(8):
                        po = pop.tile([128, TG], F32, tag="po", name="po")
                        for fc in range(4):
                            nc.tensor.matmul(po[:], wo_t[:, fc, ts(do, 128)],
                                             y_t[fc][:],
                                             start=(fc == 0), stop=(fc == 3))
                        osb = opool.tile([128, TG], F32, tag="ot", name="ot")
                        nc.vector.tensor_copy(osb[:], po[:])
                        nc.sync.dma_start(out=outT[ts(do, 128), ts(g, TG)],
                                          in_=osb[:])

    nc.finalize()
    return nc


_PROGRAM_CACHE = {}


def _detect_mode(attn_mask, b_qkv):
    m2d = np.asarray(attn_mask).reshape(T, T)
    if np.array_equal(m2d != 0, np.tril(np.ones((T, T), dtype=bool))):
        mode = "causal"
    elif np.all(m2d != 0):
        mode = "full"
    else:
        mode = "generic"
    return mode, bool(np.any(np.asarray(b_qkv) != 0.0))


def _prepare_in_maps(x, attn_mask, w_qkv, b_qkv, w_out, mode, with_qkv_bias):
    m2d = np.asarray(attn_mask).reshape(T, T)
    in_maps = []
    for c in range(N_CORES):
        b, hg = c // 2, c % 2
        sl = slice(hg * FH, (hg + 1) * FH)
        im = {
            "xT": np.ascontiguousarray(x[b].T),
            "wqT": np.ascontiguousarray(w_qkv[sl, :].T),
            "wkT": np.ascontiguousarray(w_qkv[D_MODEL:][sl, :].T),
            "wvT": np.ascontiguousarray(w_qkv[2 * D_MODEL:][sl, :].T),
            "woT": np.ascontiguousarray(w_out[:, sl].T),
        }
        if mode == "causal":
            # trineg[i, r*512 + jq] = 0 if visible else -1e30, where for
            # diagonal chunk offset r: visible iff jq >= 128*r + i
            i_idx = np.arange(TK)[:, None]
            blocks = []
            for r in range(4):
                jq = np.arange(TG)[None, :]
                blocks.append(np.where(jq >= TK * r + i_idx, 0.0, NEG))
            im["trineg"] = np.concatenate(blocks, axis=1).astype(np.float32)
        elif mode == "generic":
            im["maskTneg"] = np.where(m2d.T != 0, 0.0, NEG).astype(np.float32)
        if with_qkv_bias:
            bq = b_qkv[sl].reshape(4, 128).T          # [128, 4]
            bk = b_qkv[D_MODEL:][sl].reshape(4, 128).T
            im["bqk"] = np.ascontiguousarray(
                np.concatenate([bq, bk], axis=1)).astype(np.float32)
            bv = b_qkv[2 * D_MODEL:][sl]              # [512]
            im["bvb"] = np.broadcast_to(bv, (128, FH)).astype(np.float32).copy()
        in_maps.append(im)
    return in_maps


def _gather_output(results, b_out):
    out = np.empty((B, T, D_MODEL), dtype=np.float32)
    for b in range(B):
        acc = results[2 * b]["outT"].astype(np.float32) + \
            results[2 * b + 1]["outT"].astype(np.float32)
        out[b] = acc.T + b_out[None, :]
    return out


def kernel(x, attn_mask, w_qkv, b_qkv, w_out, b_out):
    from concourse.bass_utils import run_bass_kernel_spmd

    x = np.asarray(x, dtype=np.float32)
    w_qkv = np.asarray(w_qkv, dtype=np.float32)
    b_qkv = np.asarray(b_qkv, dtype=np.float32)
    w_out = np.asarray(w_out, dtype=np.float32)
    b_out = np.asarray(b_out, dtype=np.float32)

    mode, with_qkv_bias = _detect_mode(attn_mask, b_qkv)
    key = (mode, with_qkv_bias)
    if key not in _PROGRAM_CACHE:
        _PROGRAM_CACHE[key] = _build_program(mode, with_qkv_bias)
    nc = _PROGRAM_CACHE[key]

    in_maps = _prepare_in_maps(x, attn_mask, w_qkv, b_qkv, w_out,
                               mode, with_qkv_bias)
    res = run_bass_kernel_spmd(nc, in_maps, core_ids=list(range(N_CORES)))
    return _gather_output(res.results, b_out)
